# revision 1
# baseline (speedup 1.0000x reference)
"""Nearest-neighbor tokenizer on Trainium2: 8 NeuronCores, code-sharded.

Per token x (d=512) against codebook C [16384, 512]:
    dist^2(x,c) = ||x||^2 + ||c||^2 - 2 x.c
    id = argmin_c dist^2   if min_c dist^2 <= 900 else -1

v2 architecture (candidate search on device, exact rescore on host):
  - Shard by CODES: core g owns codes[g*2048:(g+1)*2048] and sees all
    8192 tokens (64 token tiles of 128).
  - Device computes v_c = x.c - ||c||^2/2 in ONE fp32r GEMM pass per
    tile. The -||c||^2/2 bias rides as a K=2 matmul (hi/lo split of the
    bias, hi exactly representable in f32r) that opens each PSUM
    accumulation group, so the GEMM result lands pre-biased in PSUM.
  - DVE pair-maxes the 2048 v values into 1024 (one PSUM + one
    ACT-drained SBUF operand), then top-8 + indices per token.
  - Host merges 8 cores x 8 pairs x 2 codes = 128 candidates/token and
    rescores them exactly in float64; argmin + threshold reproduce the
    reference bit-exactly as long as the true winner is among the
    candidates (fp32r noise ~2e-3 vs needing 8 closer pairs: safe).
"""

import sys

import numpy as np

try:
    import concourse.bass as _probe_bass  # noqa: F401
except Exception:  # pragma: no cover
    sys.path.insert(0, "/opt/trn_rl_repo")

B, S, D = 4, 2048, 512
C = 16384
N_CORES = 8
NTOK = B * S                   # 8192 tokens, all seen by every core
N_TILES = NTOK // 128          # 64 token tiles
G = C // N_CORES               # 2048 codes per core
KC = D // 128                  # 4 contraction chunks
NSLC = G // 512                # 4 psum bank slices
HALF = G // 2                  # 1024 pairs

_CACHE: dict = {}


def _build_program(nc=None):
    import concourse.tile as tile
    from concourse import mybir

    f32 = mybir.dt.float32
    f32r = mybir.dt.float32r
    u32 = mybir.dt.uint32
    Alu = mybir.AluOpType
    Act = mybir.ActivationFunctionType

    if nc is None:
        # Bacc: its finalize() runs the TRN2 wait-splitting compile passes
        # (plain Bass emits multi-wait DMAs that walrus codegen rejects).
        from concourse import bacc

        nc = bacc.Bacc("TRN2", target_bir_lowering=False, debug=False)

    xs_d = nc.declare_dram_parameter("xs", [128, N_TILES * D], f32, isOutput=False)
    cr_d = nc.declare_dram_parameter("cr", [128, KC * G], f32, isOutput=False)
    cb2_d = nc.declare_dram_parameter("cb2", [2, G], f32, isOutput=False)
    cval_d = nc.declare_dram_parameter("cval", [128, N_TILES * 8], f32, isOutput=True)
    cidx_d = nc.declare_dram_parameter("cidx", [128, N_TILES * 8], u32, isOutput=True)

    with tile.TileContext(nc) as tc:
        with (
            tc.tile_pool(name="const", bufs=1) as const,
            tc.tile_pool(name="work", bufs=3) as work,
            tc.tile_pool(name="psum", bufs=2, space="PSUM") as psum,
        ):
            # One-time: codes + bias to SBUF, rounded to f32r.
            crb = const.tile([128, KC * G], f32, name="crb")
            nc.sync.dma_start(crb[:], cr_d[:])
            crr = const.tile([128, KC * G], f32r, name="crr")
            nc.vector.tensor_copy(crr[:], crb[:])
            cb2b = const.tile([2, G], f32, name="cb2b")
            nc.sync.dma_start(cb2b[:], cb2_d[:])
            cb2r = const.tile([2, G], f32r, name="cb2r")
            nc.vector.tensor_copy(cb2r[:], cb2b[:])
            onesb = const.tile([2, 128], f32, name="onesb")
            nc.vector.memset(onesb[:], 1.0)
            onesr = const.tile([2, 128], f32r, name="onesr")
            nc.vector.tensor_copy(onesr[:], onesb[:])

            cval = const.tile([128, N_TILES * 8], f32, name="cval")
            cidx = const.tile([128, N_TILES * 8], u32, name="cidx")

            for t in range(N_TILES):
                xsb = work.tile([128, D], f32, name="xsb")
                nc.sync.dma_start(xsb[:], xs_d[:, t * D:(t + 1) * D])
                xr = work.tile([128, D], f32r, name="xr")
                nc.scalar.activation(xr[:], xsb[:], Act.Copy)

                ps = psum.tile([128, G], f32, name="ps")
                for s in range(NSLC):
                    nc.tensor.matmul(
                        ps[:, s * 512:(s + 1) * 512],
                        onesr[:],
                        cb2r[:, s * 512:(s + 1) * 512],
                        start=True,
                        stop=False,
                    )
                    for k in range(KC):
                        nc.tensor.matmul(
                            ps[:, s * 512:(s + 1) * 512],
                            xr[:, k * 128:(k + 1) * 128],
                            crr[:, k * G + s * 512:k * G + (s + 1) * 512],
                            start=False,
                            stop=(k == KC - 1),
                        )

                h1 = work.tile([128, HALF], f32, name="h1")
                nc.scalar.activation(h1[:], ps[:, HALF:], Act.Copy)
                pm = work.tile([128, HALF], f32, name="pm")
                nc.vector.tensor_tensor(pm[:], ps[:, :HALF], h1[:], Alu.max)
                nc.vector.max(cval[:, t * 8:(t + 1) * 8], pm[:])
                nc.vector.max_index(
                    cidx[:, t * 8:(t + 1) * 8], cval[:, t * 8:(t + 1) * 8], pm[:]
                )

            nc.sync.dma_start(cval_d[:], cval[:])
            nc.sync.dma_start(cidx_d[:], cidx[:])

    return nc


def _prepare_in_maps(x: np.ndarray, codes: np.ndarray) -> list:
    x = np.ascontiguousarray(np.asarray(x, dtype=np.float32).reshape(NTOK, D))
    codes = np.ascontiguousarray(np.asarray(codes, dtype=np.float32))

    # xs[p, t*512 + k*128 + m] = x[t*128 + m, k*128 + p]  (same for all cores)
    xs = np.ascontiguousarray(
        x.reshape(N_TILES, 128, KC, 128).transpose(3, 0, 2, 1).reshape(128, -1)
    )

    in_maps = []
    for g in range(N_CORES):
        cg = codes[g * G:(g + 1) * G]  # [2048, 512]
        # cr[p, k*2048 + n] = cg[n, k*128 + p]
        cr = np.ascontiguousarray(
            cg.reshape(G, KC, 128).transpose(2, 1, 0).reshape(128, -1)
        )
        c2neg = (-0.5 * (cg.astype(np.float64) ** 2).sum(1)).astype(np.float32)
        # hi: keep top 11 mantissa bits -> exactly representable in f32r,
        # so the on-device f32r rounding of hi is the identity.
        hi = (c2neg.view(np.uint32) & np.uint32(0xFFFFF000)).view(np.float32)
        lo = (c2neg.astype(np.float64) - hi).astype(np.float32)
        cb2 = np.ascontiguousarray(np.stack([hi, lo]).astype(np.float32))
        in_maps.append({"xs": xs, "cr": cr, "cb2": cb2})
    return in_maps


def _postprocess(results: list, x: np.ndarray, codes: np.ndarray) -> np.ndarray:
    x64 = np.asarray(x, dtype=np.float64).reshape(NTOK, D)
    c64 = np.asarray(codes, dtype=np.float64)
    c2 = (c64 ** 2).sum(1)
    x2 = (x64 ** 2).sum(1)

    # cidx[g]: [128, 64*8]; token = t*128 + partition; local pair j -> codes
    # {g*2048 + j, g*2048 + j + 1024}.
    cand = np.empty((NTOK, N_CORES * 8), np.int64)
    for g in range(N_CORES):
        ci = np.asarray(results[g]["cidx"]).astype(np.int64)
        ci = ci.reshape(128, N_TILES, 8).transpose(1, 0, 2).reshape(NTOK, 8)
        cand[:, g * 8:(g + 1) * 8] = ci + g * G
    cands = np.concatenate([cand, cand + HALF], axis=1)  # [NTOK, 128]
    cands.sort(axis=1)  # argmin tie-break: first occurrence = lowest index

    ids = np.empty(NTOK, np.int64)
    CH = 1024
    rows = np.arange(CH)
    for i in range(0, NTOK, CH):
        cc = cands[i:i + CH]
        xc = np.einsum("tkd,td->tk", c64[cc], x64[i:i + CH], optimize=True)
        d2 = np.maximum(x2[i:i + CH, None] + c2[cc] - 2.0 * xc, 0.0)
        k = d2.argmin(1)
        ids[i:i + CH] = np.where(d2[rows, k] <= 900.0, cc[rows, k], -1)
    return ids.reshape(B, S).astype(np.int32)


def kernel(x: np.ndarray, codes: np.ndarray) -> np.ndarray:
    from concourse.bass_utils import run_bass_kernel_spmd

    if "nc" not in _CACHE:
        nc = _build_program()
        nc.finalize()  # Bacc: runs wait-splitting + register allocation
        _CACHE["nc"] = nc
    in_maps = _prepare_in_maps(x, codes)
    res = run_bass_kernel_spmd(_CACHE["nc"], in_maps, list(range(N_CORES)))
    return _postprocess(res.results, x, codes)



# revision 3
# speedup vs baseline: 1.5624x; 1.5624x over previous
"""Nearest-neighbor tokenizer on Trainium2: 8 NeuronCores, code-sharded.

Per token x (d=512) against codebook C [16384, 512]:
    dist^2(x,c) = ||x||^2 + ||c||^2 - 2 x.c
    id = argmin_c dist^2   if min_c dist^2 <= 900 else -1

v6 architecture (fp8 DoubleRow candidate search, exact host rescore):
  - Shard by CODES: core g owns codes[g*2048:(g+1)*2048], sees all 8192
    tokens (64 token tiles of 128).
  - Device ranks codes by v = x.c + b where b = 256 - ||c||^2/2 (the
    global +256 shift is rank-neutral). GEMM runs in fp8e4m3 with
    perf_mode=DoubleRow (K=256 per matmul, 0.5 cycles/row): per tile
    4 PSUM slices x [bias MM (4-term fp8 decomposition of b, K=4) +
    2 main MMs].
  - PSUM exit (the bottleneck; only ACT and DVE can read PSUM):
      psL [128,512]  : DVE tensor_reduce 16-group max -> 32 slots
      psBig [128,1536]: one ACT drain -> bf16 h, then a DVE
        scalar_tensor_tensor pair-max tree (4x mode) 768/384/192/96/48
        -> 48 slots of 32 codes
  - DVE max8/max_index over the 80 slot values -> top-8 (value, slot)
    per (token, core).
  - Host keeps slots within MARGIN of the global best value and
    rescores their codes exactly in float64; argmin + threshold then
    reproduce the reference (the fp8 ranking noise is ~0.9 sigma,
    MARGIN=10 is ~8 sigma).
"""

import sys

import numpy as np

try:
    import concourse.bass as _probe_bass  # noqa: F401
except Exception:  # pragma: no cover
    sys.path.insert(0, "/opt/trn_rl_repo")

import ml_dtypes

B, S, D = 4, 2048, 512
C = 16384
N_CORES = 8
NTOK = B * S                   # 8192 tokens, all seen by every core
N_TILES = NTOK // 128          # 64 token tiles
G = C // N_CORES               # 2048 codes per core
WL = 512                       # psL piece (bank 0): reduce-16 -> 32 slots
WB = G - WL                    # 1536 psBig piece: ACT drain + DVE tree
XCH = 8                        # token tiles per xs DMA chunk
MARGIN = 10.0

FP8 = ml_dtypes.float8_e4m3

_CACHE: dict = {}


def _build_program(nc=None):
    import concourse.tile as tile
    from concourse import mybir

    f32 = mybir.dt.float32
    bf16 = mybir.dt.bfloat16
    fp8 = mybir.dt.float8e4
    u32 = mybir.dt.uint32
    Alu = mybir.AluOpType
    Act = mybir.ActivationFunctionType
    DR = mybir.MatmulPerfMode.DoubleRow
    X = mybir.AxisListType.X

    if nc is None:
        from concourse import bacc

        nc = bacc.Bacc("TRN2", target_bir_lowering=False, debug=False)

    xs_d = nc.declare_dram_parameter("xs", [128, N_TILES * 512], fp8, isOutput=False)
    cr_d = nc.declare_dram_parameter("cr", [128, 4 * G], fp8, isOutput=False)
    cb_d = nc.declare_dram_parameter("cb", [2, 2 * G], fp8, isOutput=False)
    on_d = nc.declare_dram_parameter("on", [2, 256], fp8, isOutput=False)
    cval_d = nc.declare_dram_parameter("cval", [128, N_TILES * 8], f32, isOutput=True)
    cidx_d = nc.declare_dram_parameter("cidx", [128, N_TILES * 8], u32, isOutput=True)

    with tile.TileContext(nc) as tc:
        with (
            tc.tile_pool(name="const", bufs=1) as const,
            tc.tile_pool(name="xch", bufs=2) as xch,
            tc.tile_pool(name="work", bufs=2) as work,
            tc.tile_pool(name="psl", bufs=2, space="PSUM") as psl,
            tc.tile_pool(name="psb", bufs=2, space="PSUM") as psb,
        ):
            cr = const.tile([128, 2, 2, G], fp8, name="cr")
            nc.sync.dma_start(cr[:], cr_d[:])
            cb = const.tile([2, 2, G], fp8, name="cb")
            nc.sync.dma_start(cb[:], cb_d[:])
            on = const.tile([2, 2, 128], fp8, name="on")
            nc.sync.dma_start(on[:], on_d[:])

            cval = const.tile([128, N_TILES * 8], f32, name="cval")
            cidx = const.tile([128, N_TILES * 8], u32, name="cidx")

            for c8 in range(N_TILES // XCH):
                xc = xch.tile([128, XCH, 2, 2, 128], fp8, name="xc")
                nc.sync.dma_start(
                    xc[:], xs_d[:, c8 * XCH * 512:(c8 + 1) * XCH * 512]
                )
                for k in range(XCH):
                    t = c8 * XCH + k
                    pl = psl.tile([128, WL], f32, name="pl")
                    pb = psb.tile([128, WB], f32, name="pb")
                    for s in range(4):
                        out = pl[:] if s == 0 else pb[:, (s - 1) * 512:s * 512]
                        nc.tensor.matmul(
                            out, on[:], cb[:, :, s * 512:(s + 1) * 512],
                            start=True, stop=False, perf_mode=DR,
                        )
                        for m in range(2):
                            nc.tensor.matmul(
                                out, xc[:, k, m],
                                cr[:, m, :, s * 512:(s + 1) * 512],
                                start=False, stop=(m == 1), perf_mode=DR,
                            )

                    mfin = work.tile([128, 80], bf16, name="mfin")
                    nc.vector.tensor_reduce(
                        mfin[:, 0:32],
                        pl[:].rearrange("p (a b) -> p a b", b=16),
                        X, Alu.max,
                    )
                    h = work.tile([128, WB], bf16, name="h")
                    nc.scalar.activation(h[:], pb[:], Act.Copy)
                    t1 = work.tile([128, 768], bf16, name="t1")
                    nc.vector.scalar_tensor_tensor(
                        t1[:], h[:, 0:768], 0.0, h[:, 768:1536], Alu.add, Alu.max
                    )
                    t2 = work.tile([128, 384], bf16, name="t2")
                    nc.vector.scalar_tensor_tensor(
                        t2[:], t1[:, 0:384], 0.0, t1[:, 384:768], Alu.add, Alu.max
                    )
                    t3 = work.tile([128, 192], bf16, name="t3")
                    nc.vector.scalar_tensor_tensor(
                        t3[:], t2[:, 0:192], 0.0, t2[:, 192:384], Alu.add, Alu.max
                    )
                    t4 = work.tile([128, 96], bf16, name="t4")
                    nc.vector.scalar_tensor_tensor(
                        t4[:], t3[:, 0:96], 0.0, t3[:, 96:192], Alu.add, Alu.max
                    )
                    nc.vector.scalar_tensor_tensor(
                        mfin[:, 32:80], t4[:, 0:48], 0.0, t4[:, 48:96],
                        Alu.add, Alu.max,
                    )
                    nc.vector.max(cval[:, t * 8:(t + 1) * 8], mfin[:])
                    nc.vector.max_index(
                        cidx[:, t * 8:(t + 1) * 8], cval[:, t * 8:(t + 1) * 8],
                        mfin[:],
                    )

            nc.sync.dma_start(cval_d[:], cval[:])
            nc.sync.dma_start(cidx_d[:], cidx[:])

    return nc


def _slot_cols() -> list:
    """slot j -> np.array of tile-local psum columns (code ids within the
    core's 2048-code shard)."""
    cols = []
    for j in range(32):                      # psL reduce-16 slots
        cols.append(np.arange(16 * j, 16 * j + 16))
    # h-tree: h index i <-> psum col 512 + i
    lvl = [np.array([i, i + 768]) for i in range(768)]          # t1
    lvl = [np.concatenate([lvl[i], lvl[i + 384]]) for i in range(384)]  # t2
    lvl = [np.concatenate([lvl[i], lvl[i + 192]]) for i in range(192)]  # t3
    lvl = [np.concatenate([lvl[i], lvl[i + 96]]) for i in range(96)]    # t4
    lvl = [np.concatenate([lvl[i], lvl[i + 48]]) for i in range(48)]    # mfin
    for k in range(48):
        cols.append(np.sort(lvl[k]) + WL)
    return cols


def _prepare_in_maps(x: np.ndarray, codes: np.ndarray) -> list:
    x = np.ascontiguousarray(np.asarray(x, dtype=np.float32).reshape(NTOK, D))
    codes = np.ascontiguousarray(np.asarray(codes, dtype=np.float32))
    x8 = x.astype(FP8)
    c8 = codes.astype(FP8)

    # xs[p, t, m, i, tok] = x8[t*128+tok, m*256+i*128+p]
    xs = np.ascontiguousarray(
        x8.reshape(N_TILES, 128, 2, 2, 128).transpose(4, 0, 2, 3, 1)
    ).reshape(128, -1)
    on = np.ones((2, 256), dtype=FP8)

    in_maps = []
    for g in range(N_CORES):
        cg8 = c8[g * G:(g + 1) * G]          # [2048, 512] fp8
        # cr[p, m, i, c] = cg8[c, m*256+i*128+p]
        cr = np.ascontiguousarray(
            cg8.reshape(G, 2, 2, 128).transpose(3, 1, 2, 0)
        ).reshape(128, -1)
        cg64 = codes[g * G:(g + 1) * G].astype(np.float64)
        b = 256.0 - 0.5 * (cg64 ** 2).sum(1)  # [-64, 64]-ish, rank-neutral shift
        terms = []
        r = b.copy()
        for _ in range(4):
            tq = r.astype(FP8)
            terms.append(tq)
            r = r - tq.astype(np.float64)
        cb = np.ascontiguousarray(np.stack(terms).reshape(2, 2 * G))
        in_maps.append({"xs": xs, "cr": cr, "cb": cb, "on": on})
    return in_maps


def _postprocess(results: list, x: np.ndarray, codes: np.ndarray) -> np.ndarray:
    x64 = np.asarray(x, dtype=np.float64).reshape(NTOK, D)
    c64 = np.asarray(codes, dtype=np.float64)
    c2 = (c64 ** 2).sum(1)
    x2 = (x64 ** 2).sum(1)

    # [NTOK, N_CORES, 8] top-8 slot values / slot ids per core
    vals = np.empty((NTOK, N_CORES, 8), np.float64)
    slots = np.empty((NTOK, N_CORES, 8), np.int64)
    for g in range(N_CORES):
        cv = np.asarray(results[g]["cval"], np.float64)
        ci = np.asarray(results[g]["cidx"]).astype(np.int64)
        vals[:, g, :] = cv.reshape(128, N_TILES, 8).transpose(1, 0, 2).reshape(NTOK, 8)
        slots[:, g, :] = ci.reshape(128, N_TILES, 8).transpose(1, 0, 2).reshape(NTOK, 8)

    best = vals.reshape(NTOK, -1).max(1)
    keep = vals >= (best[:, None, None] - MARGIN)
    tk, gk, rk = np.nonzero(keep)
    sk = slots[tk, gk, rk]

    slot_cols = _slot_cols()                 # 80 slots; 32x16 cols + 48x32 cols
    small = sk < 32
    ids = np.full(NTOK, -1, np.int64)
    bestd = np.full(NTOK, np.inf, np.float64)

    cmap16 = np.stack(slot_cols[:32])        # [32, 16]
    cmap32 = np.stack(slot_cols[32:])        # [48, 32]
    for mask, width in ((small, 16), (~small, 32)):
        if not mask.any():
            continue
        tkm, gkm, skm = tk[mask], gk[mask], sk[mask]
        if width == 16:
            cand = cmap16[skm] + (gkm * G)[:, None]          # [K, 16]
        else:
            cand = cmap32[skm - 32] + (gkm * G)[:, None]     # [K, 32]
        CH = 65536
        for i in range(0, len(tkm), CH):
            tc_ = tkm[i:i + CH]
            cc = cand[i:i + CH]
            xc = np.einsum("kcd,kd->kc", c64[cc], x64[tc_], optimize=True)
            d2 = np.maximum(x2[tc_][:, None] + c2[cc] - 2.0 * xc, 0.0)
            # fold into per-token running argmin with lowest-id tie-break
            tflat = np.repeat(tc_, cc.shape[1])
            dflat = d2.ravel()
            cflat = cc.ravel()
            order = np.lexsort((cflat, dflat, tflat))
            to, do_, co = tflat[order], dflat[order], cflat[order]
            first = np.unique(to, return_index=True)[1]
            tsel, dsel, csel = to[first], do_[first], co[first]
            upd = (dsel < bestd[tsel]) | (
                (dsel == bestd[tsel]) & (csel < ids[tsel])
            )
            bestd[tsel[upd]] = dsel[upd]
            ids[tsel[upd]] = csel[upd]

    ids = np.where(bestd <= 900.0, ids, -1)
    return ids.reshape(B, S).astype(np.int32)


def kernel(x: np.ndarray, codes: np.ndarray) -> np.ndarray:
    from concourse.bass_utils import run_bass_kernel_spmd

    if "nc" not in _CACHE:
        nc = _build_program()
        nc.finalize()
        _CACHE["nc"] = nc
    in_maps = _prepare_in_maps(x, codes)
    res = run_bass_kernel_spmd(_CACHE["nc"], in_maps, list(range(N_CORES)))
    return _postprocess(res.results, x, codes)


# revision 25
# speedup vs baseline: 2.2697x; 1.4527x over previous
"""Nearest-neighbor tokenizer on Trainium2: 8 NeuronCores, code-sharded.

Per token x (d=512) against codebook C [16384, 512]:
    dist^2(x,c) = ||x||^2 + ||c||^2 - 2 x.c
    id = argmin_c dist^2   if min_c dist^2 <= 900 else -1

v6 architecture (fp8 DoubleRow candidate search, exact host rescore):
  - Shard by CODES: core g owns codes[g*2048:(g+1)*2048], sees all 8192
    tokens (64 token tiles of 128).
  - Device ranks codes by v = x.c + b where b = 256 - ||c||^2/2 (the
    global +256 shift is rank-neutral). GEMM runs in fp8e4m3 with
    perf_mode=DoubleRow (K=256 per matmul, 0.5 cycles/row): per tile
    4 PSUM slices x [bias MM (4-term fp8 decomposition of b, K=4) +
    2 main MMs].
  - PSUM exit (the bottleneck; only ACT and DVE can read PSUM):
      psL [128,512]  : DVE tensor_reduce 16-group max -> 32 slots
      psBig [128,1536]: one ACT drain -> bf16 h, then a DVE
        scalar_tensor_tensor pair-max tree (4x mode) 768/384/192/96/48
        -> 48 slots of 32 codes
  - DVE max8/max_index over the 80 slot values -> top-8 (value, slot)
    per (token, core).
  - Host keeps slots within MARGIN of the global best value and
    rescores their codes exactly in float64; argmin + threshold then
    reproduce the reference (the fp8 ranking noise is ~0.9 sigma,
    MARGIN=10 is ~8 sigma).
"""

import sys

import numpy as np

try:
    import concourse.bass as _probe_bass  # noqa: F401
except Exception:  # pragma: no cover
    sys.path.insert(0, "/opt/trn_rl_repo")

import ml_dtypes

B, S, D = 4, 2048, 512
C = 16384
N_CORES = 8
NTOK = B * S                   # 8192 tokens, all seen by every core
N_TILES = NTOK // 128          # 64 token tiles
G = C // N_CORES               # 2048 codes per core
XCH = 8                        # token tiles per xs DMA chunk
MARGIN = 10.0
NSLOT = 64                     # tree slots of 32 codes each

FP8 = ml_dtypes.float8_e4m3

_CACHE: dict = {}


def _build_program(nc=None):
    import concourse.tile as tile
    from concourse import mybir

    f32 = mybir.dt.float32
    bf16 = mybir.dt.bfloat16
    fp8 = mybir.dt.float8e4
    u32 = mybir.dt.uint32
    Alu = mybir.AluOpType
    Act = mybir.ActivationFunctionType
    DR = mybir.MatmulPerfMode.DoubleRow
    X = mybir.AxisListType.X

    if nc is None:
        from concourse import bacc

        nc = bacc.Bacc("TRN2", target_bir_lowering=False, debug=False)

    xs_d = nc.declare_dram_parameter("xs", [128, N_TILES * 512], fp8, isOutput=False)
    cr_d = nc.declare_dram_parameter("cr", [128, 4 * G], fp8, isOutput=False)
    cb_d = nc.declare_dram_parameter("cb", [2, 2 * G], fp8, isOutput=False)
    on_d = nc.declare_dram_parameter("on", [2, 256], fp8, isOutput=False)
    cval_d = nc.declare_dram_parameter("cval", [128, N_TILES * 8], f32, isOutput=True)
    cidx_d = nc.declare_dram_parameter("cidx", [128, N_TILES * 8], u32, isOutput=True)

    with tile.TileContext(nc) as tc:
        with (
            tc.tile_pool(name="const", bufs=1) as const,
            tc.tile_pool(name="xch", bufs=2) as xch,
            tc.tile_pool(name="work", bufs=2) as work,
            tc.tile_pool(name="psum", bufs=2, space="PSUM") as psum,
        ):
            # cb+on first (tiny, DVE queue) so the bias MMs can issue early;
            # cr arrives in per-slice chunks (ACT queue) overlapping xs (SP).
            cb = const.tile([2, 2, G], fp8, name="cb")
            nc.gpsimd.dma_start(cb[:], cb_d[:])
            on = const.tile([2, 2, 128], fp8, name="on")
            nc.gpsimd.dma_start(on[:], on_d[:])
            cr = const.tile([128, 2, 2, G], fp8, name="cr")
            cr_v = cr_d[:].rearrange("p (m i c) -> p m i c", m=2, i=2)
            for s in range(4):
                nc.scalar.dma_start(
                    cr[:, :, :, s * 512:(s + 1) * 512],
                    cr_v[:, :, :, s * 512:(s + 1) * 512],
                )

            cval = const.tile([128, N_TILES * 8], f32, name="cval")
            cidx = const.tile([128, N_TILES * 8], u32, name="cidx")

            # preload the ACT function table off the critical path
            warm = const.tile([2, 8], f32, name="warm")
            nc.vector.memset(warm[:], 0.0)
            warmo = const.tile([2, 8], bf16, name="warmo")
            nc.scalar.activation(warmo[:], warm[:], Act.Copy)

            for c8 in range(N_TILES // XCH):
                xc = xch.tile([128, XCH, 2, 2, 128], fp8, name="xc")
                nc.sync.dma_start(
                    xc[:], xs_d[:, c8 * XCH * 512:(c8 + 1) * XCH * 512]
                )
                for k in range(XCH):
                    t = c8 * XCH + k
                    ps = psum.tile([128, G], f32, name="ps")
                    for s in range(4):
                        out = ps[:, s * 512:(s + 1) * 512]
                        nc.tensor.matmul(
                            out, on[:], cb[:, :, s * 512:(s + 1) * 512],
                            start=True, stop=False, perf_mode=DR,
                        )
                        for m in range(2):
                            nc.tensor.matmul(
                                out, xc[:, k, m],
                                cr[:, m, :, s * 512:(s + 1) * 512],
                                start=False, stop=(m == 1), perf_mode=DR,
                            )

                    mfin = work.tile([128, NSLOT], bf16, name="mfin")
                    h = work.tile([128, G], bf16, name="h")
                    nc.scalar.activation(h[:], ps[:], Act.Copy)
                    t1 = work.tile([128, 1024], bf16, name="t1")
                    nc.vector.tensor_tensor(
                        t1[:], h[:, 0:1024], h[:, 1024:2048], Alu.max
                    )
                    t2 = work.tile([128, 512], bf16, name="t2")
                    nc.vector.tensor_tensor(
                        t2[:], t1[:, 0:512], t1[:, 512:1024], Alu.max
                    )
                    t3 = work.tile([128, 256], bf16, name="t3")
                    nc.vector.tensor_tensor(
                        t3[:], t2[:, 0:256], t2[:, 256:512], Alu.max
                    )
                    t4 = work.tile([128, 128], bf16, name="t4")
                    nc.vector.tensor_tensor(
                        t4[:], t3[:, 0:128], t3[:, 128:256], Alu.max
                    )
                    nc.vector.tensor_tensor(
                        mfin[:], t4[:, 0:64], t4[:, 64:128], Alu.max
                    )
                    nc.vector.max(cval[:, t * 8:(t + 1) * 8], mfin[:])
                    nc.vector.max_index(
                        cidx[:, t * 8:(t + 1) * 8], cval[:, t * 8:(t + 1) * 8],
                        mfin[:],
                    )

            nc.sync.dma_start(cval_d[:], cval[:])
            nc.gpsimd.dma_start(cidx_d[:], cidx[:])

    return nc


def _slot_cols() -> list:
    """slot j -> np.array of tile-local psum columns (code ids within the
    core's 2048-code shard)."""
    lvl = [np.array([i, i + 1024]) for i in range(1024)]                # t1
    lvl = [np.concatenate([lvl[i], lvl[i + 512]]) for i in range(512)]  # t2
    lvl = [np.concatenate([lvl[i], lvl[i + 256]]) for i in range(256)]  # t3
    lvl = [np.concatenate([lvl[i], lvl[i + 128]]) for i in range(128)]  # t4
    lvl = [np.concatenate([lvl[i], lvl[i + 64]]) for i in range(64)]    # mfin
    return [np.sort(lvl[k]) for k in range(NSLOT)]


def _prepare_in_maps(x: np.ndarray, codes: np.ndarray) -> list:
    x = np.ascontiguousarray(np.asarray(x, dtype=np.float32).reshape(NTOK, D))
    codes = np.ascontiguousarray(np.asarray(codes, dtype=np.float32))
    x8 = x.astype(FP8)
    c8 = codes.astype(FP8)

    # xs[p, t, m, i, tok] = x8[t*128+tok, m*256+i*128+p]
    xs = np.ascontiguousarray(
        x8.reshape(N_TILES, 128, 2, 2, 128).transpose(4, 0, 2, 3, 1)
    ).reshape(128, -1)
    on = np.ones((2, 256), dtype=FP8)

    in_maps = []
    for g in range(N_CORES):
        cg8 = c8[g * G:(g + 1) * G]          # [2048, 512] fp8
        # cr[p, m, i, c] = cg8[c, m*256+i*128+p]
        cr = np.ascontiguousarray(
            cg8.reshape(G, 2, 2, 128).transpose(3, 1, 2, 0)
        ).reshape(128, -1)
        cg64 = codes[g * G:(g + 1) * G].astype(np.float64)
        b = 256.0 - 0.5 * (cg64 ** 2).sum(1)  # [-64, 64]-ish, rank-neutral shift
        terms = []
        r = b.copy()
        for _ in range(4):
            tq = r.astype(FP8)
            terms.append(tq)
            r = r - tq.astype(np.float64)
        cb = np.ascontiguousarray(np.stack(terms).reshape(2, 2 * G))
        in_maps.append({"xs": xs, "cr": cr, "cb": cb, "on": on})
    return in_maps


def _postprocess(results: list, x: np.ndarray, codes: np.ndarray) -> np.ndarray:
    x64 = np.asarray(x, dtype=np.float64).reshape(NTOK, D)
    c64 = np.asarray(codes, dtype=np.float64)
    c2 = (c64 ** 2).sum(1)
    x2 = (x64 ** 2).sum(1)

    # [NTOK, N_CORES, 8] top-8 slot values / slot ids per core
    vals = np.empty((NTOK, N_CORES, 8), np.float64)
    slots = np.empty((NTOK, N_CORES, 8), np.int64)
    for g in range(N_CORES):
        cv = np.asarray(results[g]["cval"], np.float64)
        ci = np.asarray(results[g]["cidx"]).astype(np.int64)
        vals[:, g, :] = cv.reshape(128, N_TILES, 8).transpose(1, 0, 2).reshape(NTOK, 8)
        slots[:, g, :] = ci.reshape(128, N_TILES, 8).transpose(1, 0, 2).reshape(NTOK, 8)

    best = vals.reshape(NTOK, -1).max(1)
    keep = vals >= (best[:, None, None] - MARGIN)
    tk, gk, rk = np.nonzero(keep)
    sk = slots[tk, gk, rk]

    slot_cols = _slot_cols()                 # NSLOT x 32 cols
    ids = np.full(NTOK, -1, np.int64)
    bestd = np.full(NTOK, np.inf, np.float64)

    cmap = np.stack(slot_cols)               # [NSLOT, 32]
    if True:
        tkm, gkm, skm = tk, gk, sk
        cand = cmap[skm] + (gkm * G)[:, None]                # [K, 32]
        CH = 65536
        for i in range(0, len(tkm), CH):
            tc_ = tkm[i:i + CH]
            cc = cand[i:i + CH]
            xc = np.einsum("kcd,kd->kc", c64[cc], x64[tc_], optimize=True)
            d2 = np.maximum(x2[tc_][:, None] + c2[cc] - 2.0 * xc, 0.0)
            # fold into per-token running argmin with lowest-id tie-break
            tflat = np.repeat(tc_, cc.shape[1])
            dflat = d2.ravel()
            cflat = cc.ravel()
            order = np.lexsort((cflat, dflat, tflat))
            to, do_, co = tflat[order], dflat[order], cflat[order]
            first = np.unique(to, return_index=True)[1]
            tsel, dsel, csel = to[first], do_[first], co[first]
            upd = (dsel < bestd[tsel]) | (
                (dsel == bestd[tsel]) & (csel < ids[tsel])
            )
            bestd[tsel[upd]] = dsel[upd]
            ids[tsel[upd]] = csel[upd]

    ids = np.where(bestd <= 900.0, ids, -1)
    return ids.reshape(B, S).astype(np.int32)


def kernel(x: np.ndarray, codes: np.ndarray) -> np.ndarray:
    from concourse.bass_utils import run_bass_kernel_spmd

    if "nc" not in _CACHE:
        nc = _build_program()
        nc.finalize()
        _CACHE["nc"] = nc
    in_maps = _prepare_in_maps(x, codes)
    res = run_bass_kernel_spmd(_CACHE["nc"], in_maps, list(range(N_CORES)))
    return _postprocess(res.results, x, codes)


# revision 34
# speedup vs baseline: 2.2742x; 1.0020x over previous
"""Nearest-neighbor tokenizer on Trainium2: 8 NeuronCores, code-sharded.

Per token x (d=512) against codebook C [16384, 512]:
    dist^2(x,c) = ||x||^2 + ||c||^2 - 2 x.c
    id = argmin_c dist^2   if min_c dist^2 <= 900 else -1

v6 architecture (fp8 DoubleRow candidate search, exact host rescore):
  - Shard by CODES: core g owns codes[g*2048:(g+1)*2048], sees all 8192
    tokens (64 token tiles of 128).
  - Device ranks codes by v = x.c + b where b = 256 - ||c||^2/2 (the
    global +256 shift is rank-neutral). GEMM runs in fp8e4m3 with
    perf_mode=DoubleRow (K=256 per matmul, 0.5 cycles/row): per tile
    4 PSUM slices x [bias MM (4-term fp8 decomposition of b, K=4) +
    2 main MMs].
  - PSUM exit (the bottleneck; only ACT and DVE can read PSUM):
      psL [128,512]  : DVE tensor_reduce 16-group max -> 32 slots
      psBig [128,1536]: one ACT drain -> bf16 h, then a DVE
        scalar_tensor_tensor pair-max tree (4x mode) 768/384/192/96/48
        -> 48 slots of 32 codes
  - DVE max8/max_index over the 80 slot values -> top-8 (value, slot)
    per (token, core).
  - Host keeps slots within MARGIN of the global best value and
    rescores their codes exactly in float64; argmin + threshold then
    reproduce the reference (the fp8 ranking noise is ~0.9 sigma,
    MARGIN=10 is ~8 sigma).
"""

import sys

import numpy as np

try:
    import concourse.bass as _probe_bass  # noqa: F401
except Exception:  # pragma: no cover
    sys.path.insert(0, "/opt/trn_rl_repo")

import ml_dtypes

B, S, D = 4, 2048, 512
C = 16384
N_CORES = 8
NTOK = B * S                   # 8192 tokens, all seen by every core
N_TILES = NTOK // 128          # 64 token tiles
G = C // N_CORES               # 2048 codes per core
XCH = 8                        # token tiles per xs DMA chunk
MARGIN = 10.0
NSLOT = 64                     # tree slots of 32 codes each
HFO = 48 * 8                   # output split point (tiles 0:48 | 48:64)

FP8 = ml_dtypes.float8_e4m3

_CACHE: dict = {}


def _build_program(nc=None):
    import concourse.tile as tile
    from concourse import mybir

    f32 = mybir.dt.float32
    bf16 = mybir.dt.bfloat16
    fp8 = mybir.dt.float8e4
    u32 = mybir.dt.uint32
    Alu = mybir.AluOpType
    Act = mybir.ActivationFunctionType
    DR = mybir.MatmulPerfMode.DoubleRow
    X = mybir.AxisListType.X

    if nc is None:
        from concourse import bacc

        nc = bacc.Bacc("TRN2", target_bir_lowering=False, debug=False)

    xs_d = nc.declare_dram_parameter("xs", [128, N_TILES * 512], fp8, isOutput=False)
    cr_d = nc.declare_dram_parameter("cr", [128, 4 * G], fp8, isOutput=False)
    cb_d = nc.declare_dram_parameter("cb", [2, 2 * G + 256], fp8, isOutput=False)
    cval_d = nc.declare_dram_parameter("cval", [128, N_TILES * 8], f32, isOutput=True)
    cidx_d = nc.declare_dram_parameter("cidx", [128, N_TILES * 8], u32, isOutput=True)

    with tile.TileContext(nc) as tc:
        with (
            tc.tile_pool(name="const", bufs=1) as const,
            tc.tile_pool(name="xch", bufs=2) as xch,
            tc.tile_pool(name="work", bufs=2) as work,
            tc.tile_pool(name="psum", bufs=2, space="PSUM") as psum,
        ):
            # cb+on first (tiny, Pool queue) so the bias MMs can issue early;
            # cr arrives as four per-slice tiles so slice s only waits its
            # own chunk: cr0 ahead of xs chunk 0 on SP, cr1-3 on ACT queue.
            cbon = const.tile([2, 2 * G + 256], fp8, name="cbon")
            nc.gpsimd.dma_start(cbon[:], cb_d[:])
            cb = cbon[:, 0:2 * G].rearrange("p (i c) -> p i c", i=2)
            on = cbon[:, 2 * G:].rearrange("p (i c) -> p i c", i=2)
            cr_v = cr_d[:].rearrange("p (m i c) -> p m i c", m=2, i=2)
            crs = []
            for s in range(4):
                crt = const.tile([128, 2, 2, 512], fp8, name=f"cr{s}")
                eng = nc.sync if s == 0 else nc.scalar
                eng.dma_start(crt[:], cr_v[:, :, :, s * 512:(s + 1) * 512])
                crs.append(crt)

            cval = const.tile([128, N_TILES * 8], f32, name="cval")
            cidx = const.tile([128, N_TILES * 8], u32, name="cidx")

            # preload the ACT function table off the critical path
            warm = const.tile([2, 8], f32, name="warm")
            nc.vector.memset(warm[:], 0.0)
            warmo = const.tile([2, 8], bf16, name="warmo")
            nc.scalar.activation(warmo[:], warm[:], Act.Copy)

            for c8 in range(N_TILES // XCH):
                xc = xch.tile([128, XCH, 2, 2, 128], fp8, name="xc")
                if c8 == 0:
                    nc.sync.dma_start(xc[:, 0:2], xs_d[:, 0:1024])
                    nc.sync.dma_start(xc[:, 2:XCH], xs_d[:, 1024:XCH * 512])
                else:
                    nc.sync.dma_start(
                        xc[:], xs_d[:, c8 * XCH * 512:(c8 + 1) * XCH * 512]
                    )
                for k in range(XCH):
                    t = c8 * XCH + k
                    ps = psum.tile([128, G], f32, name="ps", tag="pst")
                    for s in range(4):
                        out = ps[:, s * 512:(s + 1) * 512]
                        nc.tensor.matmul(
                            out, on, cb[:, :, s * 512:(s + 1) * 512],
                            start=True, stop=False, perf_mode=DR,
                        )
                        for m in range(2):
                            nc.tensor.matmul(
                                out, xc[:, k, m],
                                crs[s][:, m],
                                start=False, stop=(m == 1), perf_mode=DR,
                            )

                    mfin = work.tile([128, NSLOT], bf16, name="mfin")
                    h = work.tile([128, G], bf16, name="h")
                    nc.scalar.activation(h[:], ps[:], Act.Copy)
                    t1 = work.tile([128, 1024], bf16, name="t1")
                    nc.vector.tensor_tensor(
                        t1[:], h[:, 0:1024], h[:, 1024:2048], Alu.max
                    )
                    t2 = work.tile([128, 512], bf16, name="t2")
                    nc.vector.tensor_tensor(
                        t2[:], t1[:, 0:512], t1[:, 512:1024], Alu.max
                    )
                    t3 = work.tile([128, 256], bf16, name="t3")
                    nc.vector.tensor_tensor(
                        t3[:], t2[:, 0:256], t2[:, 256:512], Alu.max
                    )
                    t4 = work.tile([128, 128], bf16, name="t4")
                    nc.vector.tensor_tensor(
                        t4[:], t3[:, 0:128], t3[:, 128:256], Alu.max
                    )
                    nc.vector.tensor_tensor(
                        mfin[:], t4[:, 0:64], t4[:, 64:128], Alu.max
                    )
                    nc.vector.max(cval[:, t * 8:(t + 1) * 8], mfin[:])
                    nc.vector.max_index(
                        cidx[:, t * 8:(t + 1) * 8], cval[:, t * 8:(t + 1) * 8],
                        mfin[:],
                    )
                if c8 == 5:
                    nc.sync.dma_start(cval_d[:, 0:HFO], cval[:, 0:HFO])
                    nc.scalar.dma_start(cidx_d[:, 0:HFO], cidx[:, 0:HFO])

            nc.sync.dma_start(cval_d[:, HFO:], cval[:, HFO:])
            nc.scalar.dma_start(cidx_d[:, HFO:], cidx[:, HFO:])

    return nc


def _slot_cols() -> list:
    """slot j -> np.array of tile-local psum columns (code ids within the
    core's 2048-code shard)."""
    lvl = [np.array([i, i + 1024]) for i in range(1024)]                # t1
    lvl = [np.concatenate([lvl[i], lvl[i + 512]]) for i in range(512)]  # t2
    lvl = [np.concatenate([lvl[i], lvl[i + 256]]) for i in range(256)]  # t3
    lvl = [np.concatenate([lvl[i], lvl[i + 128]]) for i in range(128)]  # t4
    lvl = [np.concatenate([lvl[i], lvl[i + 64]]) for i in range(64)]    # mfin
    return [np.sort(lvl[k]) for k in range(NSLOT)]


def _prepare_in_maps(x: np.ndarray, codes: np.ndarray) -> list:
    x = np.ascontiguousarray(np.asarray(x, dtype=np.float32).reshape(NTOK, D))
    codes = np.ascontiguousarray(np.asarray(codes, dtype=np.float32))
    x8 = x.astype(FP8)
    c8 = codes.astype(FP8)

    # xs[p, t, m, i, tok] = x8[t*128+tok, m*256+i*128+p]
    xs = np.ascontiguousarray(
        x8.reshape(N_TILES, 128, 2, 2, 128).transpose(4, 0, 2, 3, 1)
    ).reshape(128, -1)
    on = np.ones((2, 256), dtype=FP8)

    in_maps = []
    for g in range(N_CORES):
        cg8 = c8[g * G:(g + 1) * G]          # [2048, 512] fp8
        # cr[p, m, i, c] = cg8[c, m*256+i*128+p]
        cr = np.ascontiguousarray(
            cg8.reshape(G, 2, 2, 128).transpose(3, 1, 2, 0)
        ).reshape(128, -1)
        cg64 = codes[g * G:(g + 1) * G].astype(np.float64)
        b = 256.0 - 0.5 * (cg64 ** 2).sum(1)  # [-64, 64]-ish, rank-neutral shift
        terms = []
        r = b.copy()
        for _ in range(4):
            tq = r.astype(FP8)
            terms.append(tq)
            r = r - tq.astype(np.float64)
        cb = np.stack(terms).reshape(2, 2 * G)
        cb = np.ascontiguousarray(np.concatenate([cb, on], axis=1))
        in_maps.append({"xs": xs, "cr": cr, "cb": cb})
    return in_maps


def _postprocess(results: list, x: np.ndarray, codes: np.ndarray) -> np.ndarray:
    x64 = np.asarray(x, dtype=np.float64).reshape(NTOK, D)
    c64 = np.asarray(codes, dtype=np.float64)
    c2 = (c64 ** 2).sum(1)
    x2 = (x64 ** 2).sum(1)

    # [NTOK, N_CORES, 8] top-8 slot values / slot ids per core
    vals = np.empty((NTOK, N_CORES, 8), np.float64)
    slots = np.empty((NTOK, N_CORES, 8), np.int64)
    for g in range(N_CORES):
        cv = np.asarray(results[g]["cval"], np.float64)
        ci = np.asarray(results[g]["cidx"]).astype(np.int64)
        vals[:, g, :] = cv.reshape(128, N_TILES, 8).transpose(1, 0, 2).reshape(NTOK, 8)
        slots[:, g, :] = ci.reshape(128, N_TILES, 8).transpose(1, 0, 2).reshape(NTOK, 8)

    best = vals.reshape(NTOK, -1).max(1)
    keep = vals >= (best[:, None, None] - MARGIN)
    tk, gk, rk = np.nonzero(keep)
    sk = slots[tk, gk, rk]

    slot_cols = _slot_cols()                 # NSLOT x 32 cols
    ids = np.full(NTOK, -1, np.int64)
    bestd = np.full(NTOK, np.inf, np.float64)

    cmap = np.stack(slot_cols)               # [NSLOT, 32]
    if True:
        tkm, gkm, skm = tk, gk, sk
        cand = cmap[skm] + (gkm * G)[:, None]                # [K, 32]
        CH = 65536
        for i in range(0, len(tkm), CH):
            tc_ = tkm[i:i + CH]
            cc = cand[i:i + CH]
            xc = np.einsum("kcd,kd->kc", c64[cc], x64[tc_], optimize=True)
            d2 = np.maximum(x2[tc_][:, None] + c2[cc] - 2.0 * xc, 0.0)
            # fold into per-token running argmin with lowest-id tie-break
            tflat = np.repeat(tc_, cc.shape[1])
            dflat = d2.ravel()
            cflat = cc.ravel()
            order = np.lexsort((cflat, dflat, tflat))
            to, do_, co = tflat[order], dflat[order], cflat[order]
            first = np.unique(to, return_index=True)[1]
            tsel, dsel, csel = to[first], do_[first], co[first]
            upd = (dsel < bestd[tsel]) | (
                (dsel == bestd[tsel]) & (csel < ids[tsel])
            )
            bestd[tsel[upd]] = dsel[upd]
            ids[tsel[upd]] = csel[upd]

    ids = np.where(bestd <= 900.0, ids, -1)
    return ids.reshape(B, S).astype(np.int32)


def kernel(x: np.ndarray, codes: np.ndarray) -> np.ndarray:
    from concourse.bass_utils import run_bass_kernel_spmd

    if "nc" not in _CACHE:
        nc = _build_program()
        nc.finalize()
        _CACHE["nc"] = nc
    in_maps = _prepare_in_maps(x, codes)
    res = run_bass_kernel_spmd(_CACHE["nc"], in_maps, list(range(N_CORES)))
    return _postprocess(res.results, x, codes)
